# revision 25
# baseline (speedup 1.0000x reference)
"""AnatomicalContrastiveLoss on 8 trn2 NeuronCores (Bass/Tile), v2.

Sharding: core c handles (b = c//2, half = c%2); v = col*128 + p, col<1024.

Host prep (layout/dtype only): proba -> fp16 [128, 1024*16] partition-blocked;
[emb|1|y] -> fp8e4m3 packed in DoubleRow super-chunks of 14 voxel-chunks
(2x7): rhs [2,455]=7x[e|1], lhsT [2,112]=7x[y]; garr = [embT|yT] fp32 rows
for the gather (baseline-identical mechanics).

Device: w = product-reduce of fp16 proba (2-stage: x8 in fp16, x2 to fp32);
per-core top-8/partition -> rank within own 1024 -> top-128 slots -> gather
[he|lab] + w value -> pair AllGather (cheap: shared HBM domain).  Sums via
74 DoubleRow matmuls [112,455] (block-diagonal packing, counts via ones col)
-> 8-core AllReduce.  Pair-merge of the two sorted slot arrays by rank, then
the contrastive tail: Ep=exp(avg*he/tau) (DVE/Pool split + ACT exp), pair
term as 13 groups of 5 f-planes: matmuls [Mp| -ones | -EiT rank-1] -> one
Ln [100,500] per group (Ei folded into psum, no bias) -> DVE fold-reduce.
Final scalar AllReduce.
"""
import os
from contextlib import ExitStack
import numpy as np

B, C, F = 4, 16, 64
V = 262144
Vh = V // 2
NCOL = 1024            # real voxel chunks per core; v = col*128 + p
NCOLP = 1036           # padded to 74 super-chunks * 14
NSC = 74               # DoubleRow super-chunks
SCB = 1134             # fp8 bytes per partition per SC: [2,455 e|1][2,112 y]
K = 100
TAU, THETA = 0.1, 0.9
GCOLS = 80             # gather row: 64 emb + 16 y
GE = 4 * GCOLS         # gather element: 4 voxel-rows = 320 f32 = 1280B
PKN = 8448             # pair packet: [128, 66] = he 64 | lab | val
SUMN = 1040            # sums [16, 65] = [C, F | count]
NCORES = 8


def build_program(stage=None):
    from concourse import bass, bacc, tile, mybir, masks

    f32 = mybir.dt.float32
    f16 = mybir.dt.float16
    bf16 = mybir.dt.bfloat16
    f8 = mybir.dt.float8e4
    u32 = mybir.dt.uint32
    i32 = mybir.dt.int32
    i16 = mybir.dt.int16
    AF = mybir.ActivationFunctionType
    OP = mybir.AluOpType
    AX = mybir.AxisListType
    AP = bass.AP
    PM = mybir.MatmulPerfMode

    def rap(base, free_dims):
        # keep the tile's real partition dim, replace free dims
        return AP(base.tensor, base.offset, [list(base.ap[0])] + [list(d) for d in free_dims])

    STAGE = int(os.environ.get("KSTAGE", "9")) if stage is None else stage
    nc = bacc.Bacc(None, target_bir_lowering=False)
    ctx = ExitStack()

    pw = nc.dram_tensor("pw", [128, NCOL * C], f16, kind="ExternalInput")
    ye = nc.dram_tensor("ye", [128, NSC * SCB], f8, kind="ExternalInput")
    garr = nc.dram_tensor("garr", [Vh, GCOLS], f32, kind="ExternalInput")
    out = nc.dram_tensor("out", [1, 1], f32, kind="ExternalOutput")

    # internal DRAM
    idxdr = nc.dram_tensor("idxdr", [128], i16)
    valin = nc.dram_tensor("valin", [1, NCOL], f32)
    pkt = nc.dram_tensor("pkt", [1, PKN], f32)
    agp = nc.dram_tensor("agp", [2, PKN], f32)
    sumsdr = nc.dram_tensor("sumsdr", [112 * 455], f32)
    sumin = nc.dram_tensor("sumin", [1, SUMN], f32)
    sumout = nc.dram_tensor("sumout", [1, SUMN], f32)
    valsA = nc.dram_tensor("valsA", [128], f32)
    valsB = nc.dram_tensor("valsB", [128], f32)
    heflat = nc.dram_tensor("heflat", [F * K], f32)
    heflat16 = nc.dram_tensor("heflat16", [F * K], mybir.dt.bfloat16)
    labflat = nc.dram_tensor("labflat", [K], f32)
    avgflat = nc.dram_tensor("avgflat", [C * F], f32)
    eiflat = nc.dram_tensor("eiflat", [F * K], mybir.dt.bfloat16)
    lin = nc.dram_tensor("lin", [1, 8], f32)
    lout = nc.dram_tensor("lout", [1, 8], f32)

    ALL = [list(range(NCORES))]
    PAIRS = [[2 * i, 2 * i + 1] for i in range(B)]

    NREP = int(os.environ.get("KREPEAT", "1"))
    with tile.TileContext(nc) as tc:
        with (
            tc.tile_pool(name="stream", bufs=2) as stream,
            tc.tile_pool(name="persist", bufs=1) as persist,
            tc.tile_pool(name="small", bufs=2) as small,
            tc.tile_pool(name="psum", bufs=1, space="PSUM") as psum,
            tc.tile_pool(name="psx", bufs=5, space="PSUM") as psx,
        ):
          ident = persist.tile([128, 128], f32)
          masks.make_identity(nc, ident[:])
          ones128 = persist.tile([128, 1], f32)
          nc.vector.memset(ones128[:], 1.0)
          iota128 = persist.tile([128, 128], f32)
          iot1 = small.tile([128, 128], i32, tag="iot1")
          nc.gpsimd.iota(iot1[:], pattern=[[1, 128]], base=0, channel_multiplier=0)
          nc.vector.tensor_copy(iota128[:], iot1[:])
          iota16 = persist.tile([128, 16], f32)
          ioti = small.tile([128, 16], i32, tag="ioti")
          nc.gpsimd.iota(ioti[:], pattern=[[1, 16]], base=0, channel_multiplier=0)
          nc.vector.tensor_copy(iota16[:], ioti[:])
          prow = small.tile([128, 1], i32, tag="prow")
          nc.gpsimd.iota(prow[:], pattern=[[0, 1]], base=0, channel_multiplier=1)
          prowu = persist.tile([128, 1], u32)
          nc.vector.tensor_copy(prowu[:], prow[:])
          prowf = persist.tile([128, 1], f32)
          nc.vector.tensor_copy(prowf[:], prow[:])
          negones = persist.tile([C, K], bf16)
          nc.vector.memset(negones[:], -1.0)
          onesK = persist.tile([1, K], bf16)
          nc.vector.memset(onesK[:], 1.0)
          for _rep in range(NREP):
            # ---------------- pass 1: proba -> w ----------------
            w_sb = persist.tile([128, NCOL], f32)
            NTP = 4
            CT = NCOL // NTP           # 256 cols per tile
            for t in range(NTP):
                ptile = stream.tile([128, CT, C], f16, tag="p")
                q = nc.sync if t % 2 == 0 else nc.scalar
                q.dma_start(ptile[:], AP(pw, t * CT * C, [[NCOL * C, 128], [1, CT * C]]))
                # stage 1: product of 8 within each half-class-group (fp16)
                p8 = stream.tile([128, CT, 2], f16, tag="p8")
                nc.vector.tensor_reduce(p8[:], rap(ptile[:], [[16, CT], [8, 2], [1, 8]]), AX.X, OP.mult)
                # stage 2: product of the 2 halves -> fp32 w
                nc.vector.tensor_reduce(w_sb[:, t * CT:(t + 1) * CT], p8[:], AX.X, OP.mult)

            # ---------------- top-k candidates ----------------
            if STAGE >= 2:
                mx8 = persist.tile([128, 8], f32)
                nc.vector.max(mx8[:], w_sb[:])
                mi8 = persist.tile([128, 8], u32)
                nc.vector.max_index(mi8[:], mx8[:], w_sb[:])
                # v = col*128 + p
                vglob = persist.tile([128, 8], u32)
                nc.vector.tensor_scalar(vglob[:], mi8[:], 7, None, OP.logical_shift_left)
                nc.vector.tensor_tensor(vglob[:], vglob[:], rap(prowu[:], [[0, 8]]), OP.add)
                i4p = small.tile([128, 8], u32, tag="i4p")
                nc.vector.tensor_scalar(i4p[:], vglob[:], 2, None, OP.logical_shift_right)
                sbp = small.tile([128, 8], u32, tag="sbp")
                nc.vector.tensor_scalar(sbp[:], vglob[:], 3, None, OP.bitwise_and)
                # sc3[p, c8, :] = {v/4+1, v%4+1, w}
                sc3 = persist.tile([128, 8, 3], f32)
                nc.vector.tensor_copy(rap(sc3[:], [[3, 8]]), i4p[:])
                nc.vector.tensor_copy(AP(sc3.tensor, sc3[:].offset + 1, [list(sc3[:].ap[0]), [3, 8]]), sbp[:])
                nc.vector.tensor_scalar(rap(sc3[:], [[3, 8], [1, 2]]), rap(sc3[:], [[3, 8], [1, 2]]), 1.0, None, OP.add)
                nc.vector.tensor_copy(AP(sc3.tensor, sc3[:].offset + 2, [list(sc3[:].ap[0]), [3, 8]]), mx8[:])

                # rank my 1024 candidates among themselves
                nc.gpsimd.dma_start(AP(valin, 0, [[8, 128], [1, 8]]), mx8[:])
                unionw = persist.tile([128, NCOL], f32)
                nc.gpsimd.dma_start(unionw[:], AP(valin, 0, [[0, 128], [1, NCOL]]))
                rk = small.tile([128, 8], f32, tag="rk")
                geb = w_sb[:]  # dead after max/max_index; reuse as scratch
                for c8 in range(8):
                    nc.vector.tensor_scalar(geb, unionw[:], mx8[:, c8:c8 + 1], None, OP.is_ge,
                                            OP.add, accum_out=rk[:, c8:c8 + 1])
                slotf = small.tile([128, 8], f32, tag="slotf")
                nc.vector.tensor_scalar(slotf[:], rk[:], -1.0, None, OP.add)
                ps_sc = psum.tile([128, 3], f32, tag="mm")
                oh = persist.tile([128, 128], f32)
                for c8 in range(8):
                    nc.vector.tensor_scalar(oh[:], iota128[:], slotf[:, c8:c8 + 1], None, OP.is_equal)
                    nc.tensor.matmul(ps_sc[:], oh[:], sc3[:, c8, :], start=(c8 == 0), stop=(c8 == 7))
                gslot = persist.tile([128, 3], f32)
                nc.scalar.activation(gslot[:], ps_sc[:], AF.Copy)
                gidxf = small.tile([128, 1], f32, tag="gidxf")
                nc.vector.tensor_scalar(gidxf[:], gslot[:, 0:1], -1.0, 0.0, OP.add, OP.max)
                gidxi = small.tile([128, 1], i16, tag="gidxi")
                nc.vector.tensor_copy(gidxi[:], gidxf[:])
                subcol = persist.tile([128, 1], f32)
                nc.vector.tensor_scalar(subcol[:], gslot[:, 1:2], -1.0, None, OP.add)
                myval = persist.tile([128, 1], f32)
                nc.vector.tensor_copy(myval[:], gslot[:, 2:3])
                nc.gpsimd.dma_start(AP(idxdr, 0, [[1, 128]]), gidxi[:])
                gidx = small.tile([128, 8], i16, tag="gidx")
                for blk in range(8):
                    nc.gpsimd.dma_start(gidx[blk * 16:(blk + 1) * 16, :], AP(idxdr, 0, [[1, 16], [16, 8]]))

            # ---------------- gather + pair packet ----------------
            if STAGE >= 3:
                gg = persist.tile([128, 1, GE], f32)
                gin = AP(garr, 0, [[GE, Vh // 4], [1, GE]])
                nc.gpsimd.dma_gather(gg[:], gin, gidx[:], num_idxs=128, num_idxs_reg=128, elem_size=GE)
                cand = persist.tile([128, GCOLS], f32)
                ohall = small.tile([128, 4], f32, tag="ohl")
                acc = small.tile([128, GCOLS], f32, tag="acc")
                for r in range(4):
                    nc.vector.tensor_scalar(ohall[:, r:r + 1], subcol[:], float(r), None, OP.is_equal)
                nc.vector.tensor_scalar(cand[:], gg[:, 0, 0:GCOLS], ohall[:, 0:1], None, OP.mult)
                for r in range(1, 4):
                    nc.vector.tensor_scalar(acc[:], gg[:, 0, r * GCOLS:(r + 1) * GCOLS], ohall[:, r:r + 1], None, OP.mult)
                    nc.vector.tensor_tensor(cand[:], cand[:], acc[:], OP.add)
                labm = small.tile([128, 16], f32, tag="labm")
                nc.vector.tensor_tensor(labm[:], cand[:, F:GCOLS], iota16[:], OP.mult)
                labmine = persist.tile([128, 1], f32)
                nc.vector.tensor_reduce(labmine[:], labm[:], AX.X, OP.add)
                nc.sync.dma_start(AP(pkt, 0, [[66, 128], [1, F]]), cand[:, 0:F])
                nc.sync.dma_start(AP(pkt, F, [[66, 128]]), labmine[:])
                nc.sync.dma_start(AP(pkt, F + 1, [[66, 128]]), myval[:])
                nc.gpsimd.collective_compute("AllGather", OP.bypass, ins=[pkt[:]], outs=[agp[:]], replica_groups=PAIRS)

            # ---------------- ye stream: sums matmuls ----------------
            ps_sums = psum.tile([112, 455], f32, tag="sums")
            TS = [10, 10, 10, 10, 10, 10, 10, 4]   # 74 super-chunks
            sc0 = 0
            for t, tsz in enumerate(TS):
                yet = stream.tile([128, tsz, SCB], f8, tag="ye")
                q = (nc.sync, nc.scalar, nc.gpsimd)[t % 3]
                q.dma_start(yet[:], AP(ye, sc0 * SCB, [[NSC * SCB, 128], [1, tsz * SCB]]))
                for s in range(tsz):
                    base = yet[:, s, :]
                    rhs = AP(base.tensor, base.offset, [list(base.ap[0]), [455, 2], [1, 455]])
                    lhsT = AP(base.tensor, base.offset + 910, [list(base.ap[0]), [112, 2], [1, 112]])
                    nc.tensor.matmul(ps_sums[:], lhsT, rhs,
                                     start=(sc0 + s == 0), stop=(sc0 + s == NSC - 1),
                                     perf_mode=PM.DoubleRow)
                sc0 += tsz

            # extract 7 diagonal blocks -> sums_sb [16, 65] (via DRAM diag AP)
            sums_all = persist.tile([112, 455], f32)
            nc.scalar.activation(sums_all[:], ps_sums[:], AF.Copy)
            nc.sync.dma_start(AP(sumsdr, 0, [[455, 112], [1, 455]]), sums_all[:])
            sdiag = persist.tile([C, 7, F + 2], f32)
            nc.sync.dma_start(sdiag[:, :, 0:F + 1], AP(sumsdr, 0, [[455, 16], [7345, 7], [1, 65]]))
            sums_sb = persist.tile([C, F + 1], f32)
            nc.vector.tensor_reduce(sums_sb[:], rap(sdiag[:], [[1, F + 1], [F + 2, 7]]), AX.X, OP.add)
            nc.sync.dma_start(AP(sumin, 0, [[F + 1, C], [1, F + 1]]), sums_sb[:])
            nc.gpsimd.collective_compute("AllReduce", OP.add, ins=[sumin[:]], outs=[sumout[:]], replica_groups=ALL)

            # ---------------- pair merge ----------------
            if STAGE >= 4:
                candA = persist.tile([128, 66], f32, tag=f"cA{_rep % 2}")
                candB = persist.tile([128, 66], f32, tag=f"cB{_rep % 2}")
                nc.sync.dma_start(candA[:], AP(agp, 0, [[66, 128], [1, 66]]))
                nc.sync.dma_start(candB[:], AP(agp, PKN, [[66, 128], [1, 66]]))
                nc.scalar.dma_start(AP(valsA, 0, [[1, 128]]), candA[:, 65:66])
                nc.scalar.dma_start(AP(valsB, 0, [[1, 128]]), candB[:, 65:66])
                bcA = small.tile([128, 128], f32, tag="bcA")
                bcB = small.tile([128, 128], f32, tag="bcB")
                nc.scalar.dma_start(bcA[:], AP(valsA, 0, [[0, 128], [1, 128]]))
                nc.scalar.dma_start(bcB[:], AP(valsB, 0, [[0, 128], [1, 128]]))
                scr = small.tile([128, 128], f32, tag="scr")
                cntB = small.tile([128, 1], f32, tag="cntB")
                nc.vector.tensor_scalar(scr[:], bcB[:], candA[:, 65:66], None, OP.is_gt,
                                        OP.add, accum_out=cntB[:])
                cntA = small.tile([128, 1], f32, tag="cntA")
                nc.vector.tensor_scalar(scr[:], bcA[:], candB[:, 65:66], None, OP.is_ge,
                                        OP.add, accum_out=cntA[:])
                sA = small.tile([128, 1], f32, tag="sA")
                nc.vector.tensor_tensor(sA[:], prowf[:], cntB[:], OP.add)
                sB = small.tile([128, 1], f32, tag="sB")
                nc.vector.tensor_tensor(sB[:], prowf[:], cntA[:], OP.add)
                ohA = persist.tile([128, 128], f32)
                nc.vector.tensor_scalar(ohA[:], iota128[:], sA[:], None, OP.is_equal)
                ohB = small.tile([128, 128], f32, tag="ohB")
                nc.vector.tensor_scalar(ohB[:], iota128[:], sB[:], None, OP.is_equal)
                ps_m = psum.tile([128, 66], f32, tag="mm")
                nc.tensor.matmul(ps_m[:], ohA[:], candA[:], start=True, stop=False)
                nc.tensor.matmul(ps_m[:], ohB[:], candB[:], start=False, stop=True)
                merged = persist.tile([128, 66], f32, tag=f"mg{_rep % 2}")
                nc.scalar.activation(merged[:], ps_m[:], AF.Copy)
                heP = merged[0:K, 0:F]
                labP = merged[0:K, F:F + 1]
                ps_hes = psum.tile([F, K], f32, tag="mm")
                nc.tensor.transpose(ps_hes[:], heP, ident[0:K, 0:K])
                heSt16 = small.tile([F, K], bf16, tag="hest16")
                nc.scalar.activation(heSt16[:], ps_hes[:], AF.Copy)
                nc.scalar.dma_start(heflat16[:], heSt16[:])
                nc.sync.dma_start(labflat[:], labP)

            # ---------------- totals -> avg ----------------
            if STAGE >= 5:
                totf = persist.tile([C, F + 1], f32)
                nc.sync.dma_start(totf[:], AP(sumout, 0, [[F + 1, C], [1, F + 1]]))
                tot = totf[:, 0:F]
                totc = totf[:, F:F + 1]
                cmax = small.tile([C, 1], f32, tag="cmax")
                nc.vector.tensor_scalar(cmax[:], totc, 1.0, None, OP.max)
                crec = small.tile([C, 1], f32, tag="crec")
                nc.vector.reciprocal(crec[:], cmax[:])
                cgt = small.tile([C, 1], f32, tag="cgt")
                nc.vector.tensor_scalar(cgt[:], totc, 0.0, None, OP.is_gt)
                csc = small.tile([C, 1], f32, tag="csc")
                nc.vector.tensor_scalar(csc[:], crec[:], cgt[:], THETA, OP.mult, OP.mult)
                avg = persist.tile([C, F], f32)
                nc.vector.tensor_scalar(avg[:], tot, csc[:], None, OP.mult)

            # ---------------- masks ----------------
            if STAGE >= 6:
                labrep16 = small.tile([C, K], f32, tag="lr16")
                nc.sync.dma_start(labrep16[:], AP(labflat, 0, [[0, C], [1, K]]))
                iotc = small.tile([C, K], i32, tag="iotc")
                nc.gpsimd.iota(iotc[:], pattern=[[0, K]], base=0, channel_multiplier=1)
                iotcf = small.tile([C, K], f32, tag="iotcf")
                nc.vector.tensor_copy(iotcf[:], iotc[:])
                M_ck = persist.tile([C, K], f32)
                nc.vector.tensor_tensor(M_ck[:], labrep16[:], iotcf[:], OP.is_equal)
                Mp16 = persist.tile([C, K], bf16)
                nc.vector.tensor_copy(Mp16[:], M_ck[:])
                nk = small.tile([C, 1], f32, tag="nk")
                nc.vector.tensor_reduce(nk[:], M_ck[:], AX.X, OP.add)
                nk2 = small.tile([C, 1], f32, tag="nk2")
                nc.vector.tensor_tensor(nk2[:], nk[:], nk[:], OP.mult)
                den = small.tile([C, 1], f32, tag="den")
                nc.vector.tensor_scalar(den[:], nk2[:], float(F), 1.0, OP.mult, OP.max)
                wc0 = small.tile([C, 1], f32, tag="wc0")
                nc.vector.reciprocal(wc0[:], den[:])
                gnk = small.tile([C, 1], f32, tag="gnk")
                nc.vector.tensor_scalar(gnk[:], nk[:], 0.0, None, OP.is_gt)
                rhs2 = persist.tile([C, 2], f32)
                nc.vector.tensor_tensor(rhs2[:, 0:1], wc0[:], gnk[:], OP.mult)
                nc.vector.tensor_tensor(rhs2[:, 1:2], rhs2[:, 0:1], nk[:], OP.mult)

            # ---------------- E tensors + pair term + loss ----------------
            if STAGE >= 7:
                he_rep = persist.tile([C, F * K], bf16, tag=f"hr{_rep % 2}")
                nc.sync.dma_start(he_rep[:], AP(heflat16, 0, [[0, C], [1, F * K]]))
                Epf = persist.tile([C, F * K], f32)
                HFK = 24 * K  # pool does f 0..23, DVE f 24..63
                avgap0 = AP(avg.tensor, avg[:].offset, [list(avg[:].ap[0]), [1, 24], [0, K]])
                avgap1 = AP(avg.tensor, avg[:].offset + 24, [list(avg[:].ap[0]), [1, F - 24], [0, K]])
                nc.gpsimd.tensor_tensor(Epf[:, 0:HFK], he_rep[:, 0:HFK], avgap0, OP.mult)
                nc.vector.tensor_tensor(Epf[:, HFK:], he_rep[:, HFK:], avgap1, OP.mult)
                Ep = persist.tile([C, F * K], bf16, tag=f"ep{_rep % 2}")
                ECH = F * K // 4
                for ch in range(4):
                    sl = slice(ch * ECH, (ch + 1) * ECH)
                    nc.scalar.activation(Ep[:, sl], Epf[:, sl], AF.Exp, scale=1.0 / TAU)

                # Eip[i, f] = avg[lab_i, f] * he[i, f];  Ei = exp(Eip/tau)
                ps_aip = psum.tile([K, F], f32, tag="mm")
                nc.tensor.matmul(ps_aip[:], M_ck[:], avg[:], start=True, stop=True)
                Eip = persist.tile([K, F], f32, tag=f"eip{_rep % 2}")
                nc.vector.tensor_tensor(Eip[:], ps_aip[:], heP, OP.mult)
                Ei = persist.tile([K, F], f32, tag=f"ei{_rep % 2}")
                nc.scalar.activation(Ei[:], Eip[:], AF.Exp, scale=1.0 / TAU)
                # EiT (negated, bf16) as a [1, F*K] row for rank-1 psum folds
                ps_eit = psum.tile([F, K], f32, tag="mm")
                nc.tensor.transpose(ps_eit[:], Ei[:], ident[0:K, 0:K])
                nEiT = small.tile([F, K], bf16, tag="neit")
                nc.scalar.activation(nEiT[:], ps_eit[:], AF.Copy, scale=-1.0)
                nc.scalar.dma_start(AP(eiflat, 0, [[K, F], [1, K]]), nEiT[:])
                nEiR = persist.tile([1, F * K], bf16, tag=f"ner{_rep % 2}")
                nc.scalar.dma_start(nEiR[:], AP(eiflat, 0, [[0, 1], [1, F * K]]))

                # pair loop: 13 groups of 5 f-planes (64 = 12*5 + 4)
                pairacc = persist.tile([K, K], f32, tag=f"pa{_rep % 2}")
                mask2 = small.tile([K, K], f32, tag="mask2")
                labrepK = small.tile([K, K], f32, tag="lrK")
                nc.sync.dma_start(labrepK[:], AP(labflat, 0, [[0, K], [1, K]]))
                nc.vector.tensor_scalar(mask2[:], labrepK[:], labP, None, OP.is_equal)
                first = True
                f0 = 0
                while f0 < F:
                    gf = min(5, F - f0)
                    w = gf * K
                    ps_x = psx.tile([K, 5 * K], f32, tag="x")
                    # psum[i, (f,j)] = E[lab_i,f,j] - Esum[f,j] - Ei[i,f]
                    nc.tensor.matmul(ps_x[:, 0:w], Mp16[:], Ep[:, f0 * K:(f0 + gf) * K], start=True, stop=False)
                    nc.tensor.matmul(ps_x[:, 0:w], negones[:], Ep[:, f0 * K:(f0 + gf) * K], start=False, stop=False)
                    for g in range(gf):
                        nc.tensor.matmul(ps_x[:, g * K:(g + 1) * K],
                                         nEiR[:, (f0 + g) * K:(f0 + g + 1) * K], onesK[:],
                                         start=False, stop=(g == gf - 1))
                    termg = small.tile([K, 5 * K], f32, tag="termg")
                    nc.scalar.activation(termg[:, 0:w], ps_x[:, 0:w], AF.Ln, scale=-1.0)
                    tfold = small.tile([K, K], f32, tag="tfold")
                    if gf > 1:
                        nc.vector.tensor_reduce(tfold[:], rap(termg[:], [[1, K], [K, gf]]), AX.X, OP.add)
                    else:
                        nc.vector.tensor_copy(tfold[:], termg[:, 0:K])
                    if first:
                        nc.vector.tensor_copy(pairacc[:], tfold[:])
                        first = False
                    else:
                        nc.vector.tensor_tensor(pairacc[:], pairacc[:], tfold[:], OP.add)
                    f0 += gf

                sm = small.tile([K, K], f32, tag="sm")
                nc.vector.tensor_tensor(sm[:], pairacc[:], mask2[:], OP.mult)
                S2 = small.tile([K, 1], f32, tag="S2")
                nc.vector.tensor_reduce(S2[:], sm[:], AX.X, OP.add)

                ps_u = psum.tile([K, 2], f32, tag="mm")
                nc.tensor.matmul(ps_u[:], M_ck[:], rhs2[:], start=True, stop=True)
                U = small.tile([K, 2], f32, tag="U")
                nc.scalar.activation(U[:], ps_u[:], AF.Copy)

                li = small.tile([K, 1], f32, tag="li")
                nc.vector.tensor_reduce(li[:], Eip[:], AX.X, OP.add)

                t1 = small.tile([K, 1], f32, tag="t1")
                nc.vector.tensor_tensor(t1[:], S2[:], U[:, 0:1], OP.mult)
                t2 = small.tile([K, 1], f32, tag="t2")
                nc.vector.tensor_tensor(t2[:], li[:], U[:, 1:2], OP.mult)
                cvec = small.tile([K, 1], f32, tag="cvec")
                nc.vector.scalar_tensor_tensor(cvec[:], t2[:], -1.0 / TAU, t1[:], OP.mult, OP.add)

                ps_t = psum.tile([1, 1], f32, tag="mm")
                nc.tensor.matmul(ps_t[:], cvec[:], ones128[0:K, :], start=True, stop=True)
                lossp = small.tile([1, 8], f32, tag="lossp")
                nc.vector.memset(lossp[:], 0.0)
                nc.scalar.activation(lossp[:, 0:1], ps_t[:], AF.Copy, scale=-0.5 / B)
                nc.sync.dma_start(lin[:], lossp[:])
                nc.gpsimd.collective_compute("AllReduce", OP.add, ins=[lin[:]], outs=[lout[:]], replica_groups=ALL)
                res = small.tile([1, 1], f32, tag="res")
                nc.sync.dma_start(res[:], lout[0:1, 0:1])
                nc.sync.dma_start(out[:], res[:])

            if STAGE < 7:
                dbg = small.tile([1, 1], f32, tag="dbg")
                if STAGE == 1:
                    nc.sync.dma_start(dbg[:], AP(sumout, 0, [[1, 1]]))
                elif STAGE == 2:
                    nc.vector.tensor_copy(dbg[:], gslot[0:1, 0:1])
                elif STAGE == 3:
                    nc.sync.dma_start(dbg[:], AP(agp, 0, [[1, 1]]))
                elif STAGE == 4:
                    nc.sync.dma_start(dbg[:], AP(labflat, 0, [[1, 1]]))
                elif STAGE == 5:
                    nc.sync.dma_start(dbg[:], AP(avgflat, 0, [[1, 1]]))
                elif STAGE == 6:
                    nc.vector.tensor_copy(dbg[:], rhs2[0:1, 0:1])
                nc.sync.dma_start(out[:], dbg[:])

    nc.compile()
    ctx.close()
    return nc


def make_in_maps(proba, y, embeddings):
    from concourse import mybir
    f8np = mybir.dt.np(mybir.dt.float8e4)
    in_maps = []
    for core in range(NCORES):
        b, h = core // 2, core % 2
        sl = slice(h * Vh, (h + 1) * Vh)
        # pw: [128, NCOL*C] fp16, v = col*128 + p
        pT = proba[b, :, sl].T.reshape(NCOL, 128, C)
        pw = np.ascontiguousarray(pT.transpose(1, 0, 2)).reshape(128, NCOL * C).astype(np.float16)
        # ye: [128, NSC*SCB] fp8 packed [2,455 rhs][2,112 lhsT] per SC
        eT = np.ascontiguousarray(embeddings[b, :, sl].T)   # [Vh, 64]
        yT = np.ascontiguousarray(y[b, :, sl].T)            # [Vh, 16]
        e1 = np.concatenate([eT, np.ones((Vh, 1), np.float32)], axis=1)  # [Vh, 65]
        e1p = np.zeros((NCOLP * 128, 65), np.float32)
        e1p[:Vh] = e1
        yp = np.zeros((NCOLP * 128, C), np.float32)
        yp[:Vh] = yT
        e4 = e1p.reshape(NSC, 7, 2, 128, 65)
        y4 = yp.reshape(NSC, 7, 2, 128, C)
        rhs = e4.transpose(3, 0, 2, 1, 4).reshape(128, NSC, 910)
        lhs = y4.transpose(3, 0, 2, 1, 4).reshape(128, NSC, 224)
        yearr = np.ascontiguousarray(
            np.concatenate([rhs, lhs], axis=2)).reshape(128, NSC * SCB).astype(f8np)
        ga = np.ascontiguousarray(np.concatenate([eT, yT], axis=1))
        in_maps.append({"pw": pw, "ye": yearr, "garr": ga})
    return in_maps


_NC = None


def kernel(proba, y, embeddings):
    global _NC
    from concourse.bass_utils import run_bass_kernel_spmd

    if _NC is None:
        _NC = build_program()
    in_maps = make_in_maps(proba, y, embeddings)
    res = run_bass_kernel_spmd(_NC, in_maps, core_ids=list(range(NCORES)))
    return np.float32(res.results[0]["out"].reshape(())).reshape(())


# revision 26
# speedup vs baseline: 1.1015x; 1.1015x over previous
"""AnatomicalContrastiveLoss on 8 trn2 NeuronCores (Bass/Tile), v2.

Sharding: core c handles (b = c//2, half = c%2); v = col*128 + p, col<1024.

Host prep (layout/dtype only): proba -> fp16 [128, 1024*16] partition-blocked;
[emb|1|y] -> fp8e4m3 packed in DoubleRow super-chunks of 14 voxel-chunks
(2x7): rhs [2,455]=7x[e|1], lhsT [2,112]=7x[y]; garr = [embT|yT] fp32 rows
for the gather (baseline-identical mechanics).

Device: w = product-reduce of fp16 proba (2-stage: x8 in fp16, x2 to fp32);
per-core top-8/partition -> rank within own 1024 -> top-128 slots -> gather
[he|lab] + w value -> pair AllGather (cheap: shared HBM domain).  Sums via
74 DoubleRow matmuls [112,455] (block-diagonal packing, counts via ones col)
-> 8-core AllReduce.  Pair-merge of the two sorted slot arrays by rank, then
the contrastive tail: Ep=exp(avg*he/tau) (DVE/Pool split + ACT exp), pair
term as 13 groups of 5 f-planes: matmuls [Mp| -ones | -EiT rank-1] -> one
Ln [100,500] per group (Ei folded into psum, no bias) -> DVE fold-reduce.
Final scalar AllReduce.
"""
import os
from contextlib import ExitStack
import numpy as np

B, C, F = 4, 16, 64
V = 262144
Vh = V // 2
NCOL = 1024            # real voxel chunks per core; v = col*128 + p
NCOLP = 1036           # padded to 74 super-chunks * 14
NSC = 74               # DoubleRow super-chunks
SCB = 1134             # fp8 bytes per partition per SC: [2,455 e|1][2,112 y]
K = 100
TAU, THETA = 0.1, 0.9
GCOLS = 80             # gather row: 64 emb + 16 y
GE = 4 * GCOLS         # gather element: 4 voxel-rows = 320 f32 = 1280B
PKN = 8448             # pair packet: [128, 66] = he 64 | lab | val
SUMN = 1040            # sums [16, 65] = [C, F | count]
NCORES = 8


def build_program(stage=None):
    from concourse import bass, bacc, tile, mybir, masks

    f32 = mybir.dt.float32
    f16 = mybir.dt.float16
    bf16 = mybir.dt.bfloat16
    f8 = mybir.dt.float8e4
    u32 = mybir.dt.uint32
    i32 = mybir.dt.int32
    i16 = mybir.dt.int16
    AF = mybir.ActivationFunctionType
    OP = mybir.AluOpType
    AX = mybir.AxisListType
    AP = bass.AP
    PM = mybir.MatmulPerfMode

    def rap(base, free_dims):
        # keep the tile's real partition dim, replace free dims
        return AP(base.tensor, base.offset, [list(base.ap[0])] + [list(d) for d in free_dims])

    STAGE = int(os.environ.get("KSTAGE", "9")) if stage is None else stage
    nc = bacc.Bacc(None, target_bir_lowering=False)
    ctx = ExitStack()

    pw = nc.dram_tensor("pw", [128, NCOL * C], f16, kind="ExternalInput")
    ye = nc.dram_tensor("ye", [128, NSC * SCB], f8, kind="ExternalInput")
    garr = nc.dram_tensor("garr", [Vh, GCOLS], f32, kind="ExternalInput")
    out = nc.dram_tensor("out", [1, 1], f32, kind="ExternalOutput")

    # internal DRAM
    idxdr = nc.dram_tensor("idxdr", [128], i16)
    valin = nc.dram_tensor("valin", [1, NCOL], f32)
    pkt = nc.dram_tensor("pkt", [1, PKN], f32)
    agp = nc.dram_tensor("agp", [2, PKN], f32)
    sumsdr = nc.dram_tensor("sumsdr", [112 * 455], f32)
    sumin = nc.dram_tensor("sumin", [1, SUMN], f32)
    sumout = nc.dram_tensor("sumout", [1, SUMN], f32)
    valsA = nc.dram_tensor("valsA", [128], f32)
    valsB = nc.dram_tensor("valsB", [128], f32)
    heflat = nc.dram_tensor("heflat", [F * K], f32)
    heflat16 = nc.dram_tensor("heflat16", [F * K], mybir.dt.bfloat16)
    labflat = nc.dram_tensor("labflat", [K], f32)
    avgflat = nc.dram_tensor("avgflat", [C * F], f32)
    eiflat = nc.dram_tensor("eiflat", [F * K], mybir.dt.bfloat16)
    lin = nc.dram_tensor("lin", [1, 8], f32)
    lout = nc.dram_tensor("lout", [1, 8], f32)

    ALL = [list(range(NCORES))]
    PAIRS = [[2 * i, 2 * i + 1] for i in range(B)]

    NREP = int(os.environ.get("KREPEAT", "1"))
    with tile.TileContext(nc) as tc:
        with (
            tc.tile_pool(name="stream", bufs=3) as stream,
            tc.tile_pool(name="persist", bufs=1) as persist,
            tc.tile_pool(name="small", bufs=2) as small,
            tc.tile_pool(name="psum", bufs=1, space="PSUM") as psum,
            tc.tile_pool(name="psx", bufs=5, space="PSUM") as psx,
        ):
          ident = persist.tile([128, 128], f32)
          masks.make_identity(nc, ident[:])
          ones128 = persist.tile([128, 1], f32)
          nc.vector.memset(ones128[:], 1.0)
          iota128 = persist.tile([128, 128], f32)
          iot1 = small.tile([128, 128], i32, tag="iot1")
          nc.gpsimd.iota(iot1[:], pattern=[[1, 128]], base=0, channel_multiplier=0)
          nc.vector.tensor_copy(iota128[:], iot1[:])
          iota16 = persist.tile([128, 16], f32)
          ioti = small.tile([128, 16], i32, tag="ioti")
          nc.gpsimd.iota(ioti[:], pattern=[[1, 16]], base=0, channel_multiplier=0)
          nc.vector.tensor_copy(iota16[:], ioti[:])
          prow = small.tile([128, 1], i32, tag="prow")
          nc.gpsimd.iota(prow[:], pattern=[[0, 1]], base=0, channel_multiplier=1)
          prowu = persist.tile([128, 1], u32)
          nc.vector.tensor_copy(prowu[:], prow[:])
          prowf = persist.tile([128, 1], f32)
          nc.vector.tensor_copy(prowf[:], prow[:])
          negones = persist.tile([C, K], bf16)
          nc.vector.memset(negones[:], -1.0)
          onesK = persist.tile([1, K], bf16)
          nc.vector.memset(onesK[:], 1.0)
          for _rep in range(NREP):
            # ---------------- pass 1: proba -> w ----------------
            w_sb = persist.tile([128, NCOL], f32)
            NTP = 4
            CT = NCOL // NTP           # 256 cols per tile
            for t in range(NTP):
                ptile = stream.tile([128, CT, C], f16, tag="p")
                q = nc.sync if t % 2 == 0 else nc.scalar
                q.dma_start(ptile[:], AP(pw, t * CT * C, [[NCOL * C, 128], [1, CT * C]]))
                # stage 1: product of 8 within each half-class-group (fp16)
                p8 = stream.tile([128, CT, 2], f16, tag="p8")
                nc.vector.tensor_reduce(p8[:], rap(ptile[:], [[16, CT], [8, 2], [1, 8]]), AX.X, OP.mult)
                # stage 2: product of the 2 halves -> fp32 w
                nc.vector.tensor_reduce(w_sb[:, t * CT:(t + 1) * CT], p8[:], AX.X, OP.mult)

            # ---------------- top-k candidates ----------------
            if STAGE >= 2:
                mx8 = persist.tile([128, 8], f32)
                nc.vector.max(mx8[:], w_sb[:])
                mi8 = persist.tile([128, 8], u32)
                nc.vector.max_index(mi8[:], mx8[:], w_sb[:])
                # v = col*128 + p
                vglob = persist.tile([128, 8], u32)
                nc.vector.tensor_scalar(vglob[:], mi8[:], 7, None, OP.logical_shift_left)
                nc.vector.tensor_tensor(vglob[:], vglob[:], rap(prowu[:], [[0, 8]]), OP.add)
                i4p = small.tile([128, 8], u32, tag="i4p")
                nc.vector.tensor_scalar(i4p[:], vglob[:], 2, None, OP.logical_shift_right)
                sbp = small.tile([128, 8], u32, tag="sbp")
                nc.vector.tensor_scalar(sbp[:], vglob[:], 3, None, OP.bitwise_and)
                # sc3[p, c8, :] = {v/4+1, v%4+1, w}
                sc3 = persist.tile([128, 8, 3], f32)
                nc.vector.tensor_copy(rap(sc3[:], [[3, 8]]), i4p[:])
                nc.vector.tensor_copy(AP(sc3.tensor, sc3[:].offset + 1, [list(sc3[:].ap[0]), [3, 8]]), sbp[:])
                nc.vector.tensor_scalar(rap(sc3[:], [[3, 8], [1, 2]]), rap(sc3[:], [[3, 8], [1, 2]]), 1.0, None, OP.add)
                nc.vector.tensor_copy(AP(sc3.tensor, sc3[:].offset + 2, [list(sc3[:].ap[0]), [3, 8]]), mx8[:])

                # rank my 1024 candidates among themselves
                nc.gpsimd.dma_start(AP(valin, 0, [[8, 128], [1, 8]]), mx8[:])
                unionw = persist.tile([128, NCOL], f32)
                nc.gpsimd.dma_start(unionw[:], AP(valin, 0, [[0, 128], [1, NCOL]]))
                rk = small.tile([128, 8], f32, tag="rk")
                geb = w_sb[:]  # dead after max/max_index; reuse as scratch
                for c8 in range(8):
                    nc.vector.tensor_scalar(geb, unionw[:], mx8[:, c8:c8 + 1], None, OP.is_ge,
                                            OP.add, accum_out=rk[:, c8:c8 + 1])
                slotf = small.tile([128, 8], f32, tag="slotf")
                nc.vector.tensor_scalar(slotf[:], rk[:], -1.0, None, OP.add)
                ps_sc = psum.tile([128, 3], f32, tag="mm")
                oh = persist.tile([128, 128], f32)
                for c8 in range(8):
                    nc.vector.tensor_scalar(oh[:], iota128[:], slotf[:, c8:c8 + 1], None, OP.is_equal)
                    nc.tensor.matmul(ps_sc[:], oh[:], sc3[:, c8, :], start=(c8 == 0), stop=(c8 == 7))
                gslot = persist.tile([128, 3], f32)
                nc.scalar.activation(gslot[:], ps_sc[:], AF.Copy)
                gidxf = small.tile([128, 1], f32, tag="gidxf")
                nc.vector.tensor_scalar(gidxf[:], gslot[:, 0:1], -1.0, 0.0, OP.add, OP.max)
                gidxi = small.tile([128, 1], i16, tag="gidxi")
                nc.vector.tensor_copy(gidxi[:], gidxf[:])
                subcol = persist.tile([128, 1], f32)
                nc.vector.tensor_scalar(subcol[:], gslot[:, 1:2], -1.0, None, OP.add)
                myval = persist.tile([128, 1], f32)
                nc.vector.tensor_copy(myval[:], gslot[:, 2:3])
                nc.gpsimd.dma_start(AP(idxdr, 0, [[1, 128]]), gidxi[:])
                gidx = small.tile([128, 8], i16, tag="gidx")
                for blk in range(8):
                    nc.gpsimd.dma_start(gidx[blk * 16:(blk + 1) * 16, :], AP(idxdr, 0, [[1, 16], [16, 8]]))

            # ---------------- gather + pair packet ----------------
            if STAGE >= 3:
                gg = persist.tile([128, 1, GE], f32)
                gin = AP(garr, 0, [[GE, Vh // 4], [1, GE]])
                nc.gpsimd.dma_gather(gg[:], gin, gidx[:], num_idxs=128, num_idxs_reg=128, elem_size=GE)
                cand = persist.tile([128, GCOLS], f32)
                ohall = small.tile([128, 4], f32, tag="ohl")
                acc = small.tile([128, GCOLS], f32, tag="acc")
                for r in range(4):
                    nc.vector.tensor_scalar(ohall[:, r:r + 1], subcol[:], float(r), None, OP.is_equal)
                nc.vector.tensor_scalar(cand[:], gg[:, 0, 0:GCOLS], ohall[:, 0:1], None, OP.mult)
                for r in range(1, 4):
                    nc.vector.tensor_scalar(acc[:], gg[:, 0, r * GCOLS:(r + 1) * GCOLS], ohall[:, r:r + 1], None, OP.mult)
                    nc.vector.tensor_tensor(cand[:], cand[:], acc[:], OP.add)
                labm = small.tile([128, 16], f32, tag="labm")
                nc.vector.tensor_tensor(labm[:], cand[:, F:GCOLS], iota16[:], OP.mult)
                labmine = persist.tile([128, 1], f32)
                nc.vector.tensor_reduce(labmine[:], labm[:], AX.X, OP.add)
                nc.sync.dma_start(AP(pkt, 0, [[66, 128], [1, F]]), cand[:, 0:F])
                nc.sync.dma_start(AP(pkt, F, [[66, 128]]), labmine[:])
                nc.sync.dma_start(AP(pkt, F + 1, [[66, 128]]), myval[:])
                nc.gpsimd.collective_compute("AllGather", OP.bypass, ins=[pkt[:]], outs=[agp[:]], replica_groups=PAIRS)

            # ---------------- ye stream: sums matmuls ----------------
            ps_sums = psum.tile([112, 455], f32, tag="sums")
            TS = [10, 10, 10, 10, 10, 10, 10, 4]   # 74 super-chunks
            sc0 = 0
            for t, tsz in enumerate(TS):
                yet = stream.tile([128, tsz, SCB], f8, tag="ye")
                q = (nc.sync, nc.scalar, nc.gpsimd)[t % 3]
                q.dma_start(yet[:], AP(ye, sc0 * SCB, [[NSC * SCB, 128], [1, tsz * SCB]]))
                for s in range(tsz):
                    base = yet[:, s, :]
                    rhs = AP(base.tensor, base.offset, [list(base.ap[0]), [455, 2], [1, 455]])
                    lhsT = AP(base.tensor, base.offset + 910, [list(base.ap[0]), [112, 2], [1, 112]])
                    nc.tensor.matmul(ps_sums[:], lhsT, rhs,
                                     start=(sc0 + s == 0), stop=(sc0 + s == NSC - 1),
                                     perf_mode=PM.DoubleRow)
                sc0 += tsz

            # extract 7 diagonal blocks -> sums_sb [16, 65] (via DRAM diag AP)
            sums_all = persist.tile([112, 455], f32)
            nc.scalar.activation(sums_all[:], ps_sums[:], AF.Copy)
            nc.sync.dma_start(AP(sumsdr, 0, [[455, 112], [1, 455]]), sums_all[:])
            sdiag = persist.tile([C, 7, F + 2], f32)
            nc.sync.dma_start(sdiag[:, :, 0:F + 1], AP(sumsdr, 0, [[455, 16], [7345, 7], [1, 65]]))
            sums_sb = persist.tile([C, F + 1], f32)
            nc.vector.tensor_reduce(sums_sb[:], rap(sdiag[:], [[1, F + 1], [F + 2, 7]]), AX.X, OP.add)
            nc.sync.dma_start(AP(sumin, 0, [[F + 1, C], [1, F + 1]]), sums_sb[:])
            nc.gpsimd.collective_compute("AllReduce", OP.add, ins=[sumin[:]], outs=[sumout[:]], replica_groups=ALL)

            # ---------------- pair merge ----------------
            if STAGE >= 4:
                candA = persist.tile([128, 66], f32, tag=f"cA{_rep % 2}")
                candB = persist.tile([128, 66], f32, tag=f"cB{_rep % 2}")
                nc.sync.dma_start(candA[:], AP(agp, 0, [[66, 128], [1, 66]]))
                nc.sync.dma_start(candB[:], AP(agp, PKN, [[66, 128], [1, 66]]))
                nc.scalar.dma_start(AP(valsA, 0, [[1, 128]]), candA[:, 65:66])
                nc.scalar.dma_start(AP(valsB, 0, [[1, 128]]), candB[:, 65:66])
                bcA = small.tile([128, 128], f32, tag="bcA")
                bcB = small.tile([128, 128], f32, tag="bcB")
                nc.scalar.dma_start(bcA[:], AP(valsA, 0, [[0, 128], [1, 128]]))
                nc.scalar.dma_start(bcB[:], AP(valsB, 0, [[0, 128], [1, 128]]))
                scr = small.tile([128, 128], f32, tag="scr")
                cntB = small.tile([128, 1], f32, tag="cntB")
                nc.vector.tensor_scalar(scr[:], bcB[:], candA[:, 65:66], None, OP.is_gt,
                                        OP.add, accum_out=cntB[:])
                cntA = small.tile([128, 1], f32, tag="cntA")
                nc.vector.tensor_scalar(scr[:], bcA[:], candB[:, 65:66], None, OP.is_ge,
                                        OP.add, accum_out=cntA[:])
                sA = small.tile([128, 1], f32, tag="sA")
                nc.vector.tensor_tensor(sA[:], prowf[:], cntB[:], OP.add)
                sB = small.tile([128, 1], f32, tag="sB")
                nc.vector.tensor_tensor(sB[:], prowf[:], cntA[:], OP.add)
                ohA = persist.tile([128, 128], f32)
                nc.vector.tensor_scalar(ohA[:], iota128[:], sA[:], None, OP.is_equal)
                ohB = small.tile([128, 128], f32, tag="ohB")
                nc.vector.tensor_scalar(ohB[:], iota128[:], sB[:], None, OP.is_equal)
                ps_m = psum.tile([128, 66], f32, tag="mm")
                nc.tensor.matmul(ps_m[:], ohA[:], candA[:], start=True, stop=False)
                nc.tensor.matmul(ps_m[:], ohB[:], candB[:], start=False, stop=True)
                merged = persist.tile([128, 66], f32, tag=f"mg{_rep % 2}")
                nc.scalar.activation(merged[:], ps_m[:], AF.Copy)
                heP = merged[0:K, 0:F]
                labP = merged[0:K, F:F + 1]
                ps_hes = psum.tile([F, K], f32, tag="mm")
                nc.tensor.transpose(ps_hes[:], heP, ident[0:K, 0:K])
                heSt16 = small.tile([F, K], bf16, tag="hest16")
                nc.scalar.activation(heSt16[:], ps_hes[:], AF.Copy)
                nc.scalar.dma_start(heflat16[:], heSt16[:])
                nc.sync.dma_start(labflat[:], labP)

            # ---------------- totals -> avg ----------------
            if STAGE >= 5:
                totf = persist.tile([C, F + 1], f32)
                nc.sync.dma_start(totf[:], AP(sumout, 0, [[F + 1, C], [1, F + 1]]))
                tot = totf[:, 0:F]
                totc = totf[:, F:F + 1]
                cmax = small.tile([C, 1], f32, tag="cmax")
                nc.vector.tensor_scalar(cmax[:], totc, 1.0, None, OP.max)
                crec = small.tile([C, 1], f32, tag="crec")
                nc.vector.reciprocal(crec[:], cmax[:])
                cgt = small.tile([C, 1], f32, tag="cgt")
                nc.vector.tensor_scalar(cgt[:], totc, 0.0, None, OP.is_gt)
                csc = small.tile([C, 1], f32, tag="csc")
                nc.vector.tensor_scalar(csc[:], crec[:], cgt[:], THETA, OP.mult, OP.mult)
                avg = persist.tile([C, F], f32)
                nc.vector.tensor_scalar(avg[:], tot, csc[:], None, OP.mult)

            # ---------------- masks ----------------
            if STAGE >= 6:
                labrep16 = small.tile([C, K], f32, tag="lr16")
                nc.sync.dma_start(labrep16[:], AP(labflat, 0, [[0, C], [1, K]]))
                iotc = small.tile([C, K], i32, tag="iotc")
                nc.gpsimd.iota(iotc[:], pattern=[[0, K]], base=0, channel_multiplier=1)
                iotcf = small.tile([C, K], f32, tag="iotcf")
                nc.vector.tensor_copy(iotcf[:], iotc[:])
                M_ck = persist.tile([C, K], f32)
                nc.vector.tensor_tensor(M_ck[:], labrep16[:], iotcf[:], OP.is_equal)
                Mp16 = persist.tile([C, K], bf16)
                nc.vector.tensor_copy(Mp16[:], M_ck[:])
                nk = small.tile([C, 1], f32, tag="nk")
                nc.vector.tensor_reduce(nk[:], M_ck[:], AX.X, OP.add)
                nk2 = small.tile([C, 1], f32, tag="nk2")
                nc.vector.tensor_tensor(nk2[:], nk[:], nk[:], OP.mult)
                den = small.tile([C, 1], f32, tag="den")
                nc.vector.tensor_scalar(den[:], nk2[:], float(F), 1.0, OP.mult, OP.max)
                wc0 = small.tile([C, 1], f32, tag="wc0")
                nc.vector.reciprocal(wc0[:], den[:])
                gnk = small.tile([C, 1], f32, tag="gnk")
                nc.vector.tensor_scalar(gnk[:], nk[:], 0.0, None, OP.is_gt)
                rhs2 = persist.tile([C, 2], f32)
                nc.vector.tensor_tensor(rhs2[:, 0:1], wc0[:], gnk[:], OP.mult)
                nc.vector.tensor_tensor(rhs2[:, 1:2], rhs2[:, 0:1], nk[:], OP.mult)

            # ---------------- E tensors + pair term + loss ----------------
            if STAGE >= 7:
                he_rep = persist.tile([C, F * K], bf16, tag=f"hr{_rep % 2}")
                nc.sync.dma_start(he_rep[:], AP(heflat16, 0, [[0, C], [1, F * K]]))
                Epf = persist.tile([C, F * K], f32)
                HFK = 24 * K  # pool does f 0..23, DVE f 24..63
                avgap0 = AP(avg.tensor, avg[:].offset, [list(avg[:].ap[0]), [1, 24], [0, K]])
                avgap1 = AP(avg.tensor, avg[:].offset + 24, [list(avg[:].ap[0]), [1, F - 24], [0, K]])
                nc.gpsimd.tensor_tensor(Epf[:, 0:HFK], he_rep[:, 0:HFK], avgap0, OP.mult)
                nc.vector.tensor_tensor(Epf[:, HFK:], he_rep[:, HFK:], avgap1, OP.mult)
                Ep = persist.tile([C, F * K], bf16, tag=f"ep{_rep % 2}")
                ECH = F * K // 4
                for ch in range(4):
                    sl = slice(ch * ECH, (ch + 1) * ECH)
                    nc.scalar.activation(Ep[:, sl], Epf[:, sl], AF.Exp, scale=1.0 / TAU)

                # Eip[i, f] = avg[lab_i, f] * he[i, f];  Ei = exp(Eip/tau)
                ps_aip = psum.tile([K, F], f32, tag="mm")
                nc.tensor.matmul(ps_aip[:], M_ck[:], avg[:], start=True, stop=True)
                Eip = persist.tile([K, F], f32, tag=f"eip{_rep % 2}")
                nc.vector.tensor_tensor(Eip[:], ps_aip[:], heP, OP.mult)
                Ei = persist.tile([K, F], f32, tag=f"ei{_rep % 2}")
                nc.scalar.activation(Ei[:], Eip[:], AF.Exp, scale=1.0 / TAU)
                # EiT (negated, bf16) as a [1, F*K] row for rank-1 psum folds
                ps_eit = psum.tile([F, K], f32, tag="mm")
                nc.tensor.transpose(ps_eit[:], Ei[:], ident[0:K, 0:K])
                nEiT = small.tile([F, K], bf16, tag="neit")
                nc.scalar.activation(nEiT[:], ps_eit[:], AF.Copy, scale=-1.0)
                nc.scalar.dma_start(AP(eiflat, 0, [[K, F], [1, K]]), nEiT[:])
                nEiR = persist.tile([1, F * K], bf16, tag=f"ner{_rep % 2}")
                nc.scalar.dma_start(nEiR[:], AP(eiflat, 0, [[0, 1], [1, F * K]]))

                # pair loop: 13 groups of 5 f-planes (64 = 12*5 + 4)
                pairacc = persist.tile([K, K], f32, tag=f"pa{_rep % 2}")
                mask2 = small.tile([K, K], f32, tag="mask2")
                labrepK = small.tile([K, K], f32, tag="lrK")
                nc.sync.dma_start(labrepK[:], AP(labflat, 0, [[0, K], [1, K]]))
                nc.vector.tensor_scalar(mask2[:], labrepK[:], labP, None, OP.is_equal)
                first = True
                f0 = 0
                while f0 < F:
                    gf = min(5, F - f0)
                    w = gf * K
                    ps_x = psx.tile([K, 5 * K], f32, tag="x")
                    # psum[i, (f,j)] = E[lab_i,f,j] - Esum[f,j] - Ei[i,f]
                    nc.tensor.matmul(ps_x[:, 0:w], Mp16[:], Ep[:, f0 * K:(f0 + gf) * K], start=True, stop=False)
                    nc.tensor.matmul(ps_x[:, 0:w], negones[:], Ep[:, f0 * K:(f0 + gf) * K], start=False, stop=False)
                    for g in range(gf):
                        nc.tensor.matmul(ps_x[:, g * K:(g + 1) * K],
                                         nEiR[:, (f0 + g) * K:(f0 + g + 1) * K], onesK[:],
                                         start=False, stop=(g == gf - 1))
                    termg = small.tile([K, 5 * K], f32, tag="termg")
                    nc.scalar.activation(termg[:, 0:w], ps_x[:, 0:w], AF.Ln, scale=-1.0)
                    tfold = small.tile([K, K], f32, tag="tfold")
                    if gf > 1:
                        nc.vector.tensor_reduce(tfold[:], rap(termg[:], [[1, K], [K, gf]]), AX.X, OP.add)
                    else:
                        nc.vector.tensor_copy(tfold[:], termg[:, 0:K])
                    if first:
                        nc.vector.tensor_copy(pairacc[:], tfold[:])
                        first = False
                    else:
                        nc.vector.tensor_tensor(pairacc[:], pairacc[:], tfold[:], OP.add)
                    f0 += gf

                sm = small.tile([K, K], f32, tag="sm")
                nc.vector.tensor_tensor(sm[:], pairacc[:], mask2[:], OP.mult)
                S2 = small.tile([K, 1], f32, tag="S2")
                nc.vector.tensor_reduce(S2[:], sm[:], AX.X, OP.add)

                ps_u = psum.tile([K, 2], f32, tag="mm")
                nc.tensor.matmul(ps_u[:], M_ck[:], rhs2[:], start=True, stop=True)
                U = small.tile([K, 2], f32, tag="U")
                nc.scalar.activation(U[:], ps_u[:], AF.Copy)

                li = small.tile([K, 1], f32, tag="li")
                nc.vector.tensor_reduce(li[:], Eip[:], AX.X, OP.add)

                t1 = small.tile([K, 1], f32, tag="t1")
                nc.vector.tensor_tensor(t1[:], S2[:], U[:, 0:1], OP.mult)
                t2 = small.tile([K, 1], f32, tag="t2")
                nc.vector.tensor_tensor(t2[:], li[:], U[:, 1:2], OP.mult)
                cvec = small.tile([K, 1], f32, tag="cvec")
                nc.vector.scalar_tensor_tensor(cvec[:], t2[:], -1.0 / TAU, t1[:], OP.mult, OP.add)

                ps_t = psum.tile([1, 1], f32, tag="mm")
                nc.tensor.matmul(ps_t[:], cvec[:], ones128[0:K, :], start=True, stop=True)
                lossp = small.tile([1, 8], f32, tag="lossp")
                nc.vector.memset(lossp[:], 0.0)
                nc.scalar.activation(lossp[:, 0:1], ps_t[:], AF.Copy, scale=-0.5 / B)
                nc.sync.dma_start(lin[:], lossp[:])
                nc.gpsimd.collective_compute("AllReduce", OP.add, ins=[lin[:]], outs=[lout[:]], replica_groups=ALL)
                res = small.tile([1, 1], f32, tag="res")
                nc.sync.dma_start(res[:], lout[0:1, 0:1])
                nc.sync.dma_start(out[:], res[:])

            if STAGE < 7:
                dbg = small.tile([1, 1], f32, tag="dbg")
                if STAGE == 1:
                    nc.sync.dma_start(dbg[:], AP(sumout, 0, [[1, 1]]))
                elif STAGE == 2:
                    nc.vector.tensor_copy(dbg[:], gslot[0:1, 0:1])
                elif STAGE == 3:
                    nc.sync.dma_start(dbg[:], AP(agp, 0, [[1, 1]]))
                elif STAGE == 4:
                    nc.sync.dma_start(dbg[:], AP(labflat, 0, [[1, 1]]))
                elif STAGE == 5:
                    nc.sync.dma_start(dbg[:], AP(avgflat, 0, [[1, 1]]))
                elif STAGE == 6:
                    nc.vector.tensor_copy(dbg[:], rhs2[0:1, 0:1])
                nc.sync.dma_start(out[:], dbg[:])

    nc.compile()
    ctx.close()
    return nc


def make_in_maps(proba, y, embeddings):
    from concourse import mybir
    f8np = mybir.dt.np(mybir.dt.float8e4)
    in_maps = []
    for core in range(NCORES):
        b, h = core // 2, core % 2
        sl = slice(h * Vh, (h + 1) * Vh)
        # pw: [128, NCOL*C] fp16, v = col*128 + p
        pT = proba[b, :, sl].T.reshape(NCOL, 128, C)
        pw = np.ascontiguousarray(pT.transpose(1, 0, 2)).reshape(128, NCOL * C).astype(np.float16)
        # ye: [128, NSC*SCB] fp8 packed [2,455 rhs][2,112 lhsT] per SC
        eT = np.ascontiguousarray(embeddings[b, :, sl].T)   # [Vh, 64]
        yT = np.ascontiguousarray(y[b, :, sl].T)            # [Vh, 16]
        e1 = np.concatenate([eT, np.ones((Vh, 1), np.float32)], axis=1)  # [Vh, 65]
        e1p = np.zeros((NCOLP * 128, 65), np.float32)
        e1p[:Vh] = e1
        yp = np.zeros((NCOLP * 128, C), np.float32)
        yp[:Vh] = yT
        e4 = e1p.reshape(NSC, 7, 2, 128, 65)
        y4 = yp.reshape(NSC, 7, 2, 128, C)
        rhs = e4.transpose(3, 0, 2, 1, 4).reshape(128, NSC, 910)
        lhs = y4.transpose(3, 0, 2, 1, 4).reshape(128, NSC, 224)
        yearr = np.ascontiguousarray(
            np.concatenate([rhs, lhs], axis=2)).reshape(128, NSC * SCB).astype(f8np)
        ga = np.ascontiguousarray(np.concatenate([eT, yT], axis=1))
        in_maps.append({"pw": pw, "ye": yearr, "garr": ga})
    return in_maps


_NC = None


def kernel(proba, y, embeddings):
    global _NC
    from concourse.bass_utils import run_bass_kernel_spmd

    if _NC is None:
        _NC = build_program()
    in_maps = make_in_maps(proba, y, embeddings)
    res = run_bass_kernel_spmd(_NC, in_maps, core_ids=list(range(NCORES)))
    return np.float32(res.results[0]["out"].reshape(())).reshape(())
